# revision 4
# baseline (speedup 1.0000x reference)
"""D4 dispersion energy kernel for 8 Trainium2 NeuronCores (Bass/Tile).

Sharding: data-parallel over atom blocks. Each core owns 6250 atoms (padded
to 6272 = 128*49) and their 48-neighbor pair lists. Small species tables are
replicated. The per-pair c6 coefficient is computed via the exact
factorization
    refc6[zi,zj,a,b] = (3/pi) sum_x alpha[zi,a,x] alpha[zj,b,x] cpw[x]
 => c6ij = sum_x A_i[x] A_j[x],  A_i[x] = sum_a zeta_i[a] alphaS[Z_i,a,x]
with alphaS = alpha * sqrt(3/pi*cpw). A is computed per atom on-device,
AllGathered across the 8 cores, then gathered per pair (24 f32 records,
col 23 carries sqrt_r4r2) with batched indirect DMA (one instruction per
chunk, offsets [128, W]).

The program is kept tiny (~150 instructions): per-pair j-side species data
(rcov, en) is gathered on-device from a replicated per-atom table instead of
being host-precomputed and uploaded, pair offsets ship as uint16 and are
widened on-device, and all per-pair indirect gathers are batched. This cuts
per-call input upload from ~40MB to ~19MB and collapses the per-call walrus
compile time (the dominant steady-state cost of run_bass_kernel_spmd).
"""

import math
import numpy as np

N_ATOMS = 50000
N_NEIGH = 48
ZMAX = 87
NREF = 7
NFREQ = 23
KCN = 7
M = 8                      # cores
NLOC = N_ATOMS // M        # 6250
AB = 49                    # atom blocks per partition
NPAD = 128 * AB            # 6272 padded atoms per core
F = AB * N_NEIGH           # 2352 pair slots per partition
NPAD_ALL = M * NPAD        # 50176
SREC = 328                 # f32 elements per species record
CH = 7                     # ab-blocks per sweep chunk
NCHUNK = AB // CH          # 7
FC = CH * N_NEIGH          # 336

K2 = 4.0 / 3.0
K4 = 4.10451
K5 = 19.08857
K6 = 254.5553148552
KN = 7.5
WF = 6.0
GA = 3.0
GC = 2.0
BOHR = 0.5291772105638411
HARTREE = 27.211386024367243
C2BOHR = 1.0 / BOHR
C2EV = 0.5 * HARTREE

_CACHE = {}
LAST_RESULTS = None  # BassKernelResults of the most recent run (for test.py)
LAST_RUN_S = None    # wall seconds of the device dispatch+run (for test.py)


def _sp(x):
    return np.float32(np.log1p(np.exp(np.float64(x))))


def _host_tables(inp):
    """Species-level host prep (O(87) tables only)."""
    f32 = np.float32
    scaleq = _sp(inp["scaleq_raw"])
    refsys = np.asarray(inp["refsys"]).astype(np.int64)
    zeff = np.asarray(inp["zeff"], f32)
    refh = np.asarray(inp["refh"], f32)
    sscale = np.asarray(inp["sscale"], f32)
    secaiw = np.asarray(inp["secaiw"], f32)
    gam = np.asarray(inp["gam"], f32)
    ascale = np.asarray(inp["ascale"], f32)
    alphaiw = np.asarray(inp["alphaiw"], f32)
    hcount = np.asarray(inp["hcount"], f32)
    cpw = np.asarray(inp["casimir_polder_weights"], f32)

    iz = zeff[refsys]
    qmod = iz + refh * scaleq
    qmod_ = np.where(qmod > 1e-8, qmod, f32(1.0))
    zeta_t = np.where(
        qmod > 1e-8,
        np.exp(f32(GA) * (1.0 - np.exp(gam[refsys] * f32(GC) * (1.0 - iz / qmod_)))),
        f32(math.exp(GA)),
    ).astype(f32)
    asec = (sscale[refsys][..., None] * secaiw[refsys] * zeta_t[..., None]).astype(f32)
    alpha = np.maximum(ascale[..., None] * (alphaiw - hcount[..., None] * asec), 0.0)
    alphaS = (alpha * np.sqrt(3.0 / np.pi * cpw)[None, None, :]).astype(f32)

    spec = np.zeros((ZMAX, SREC), f32)
    nm = np.asarray(inp["ncount_mask"], f32).reshape(ZMAX, 49)
    nw = np.asarray(inp["ncount_weight"], f32).reshape(ZMAX, 49)
    cn = np.asarray(inp["cn"], f32).reshape(ZMAX, 49)
    spec[:, 0:49] = nm
    spec[:, 49:98] = -f32(WF) * nw
    spec[:, 98:147] = cn
    spec[:, 147:154] = np.asarray(inp["fixgweights"], f32)
    spec[:, 154:161] = np.asarray(inp["refq"], f32) * scaleq
    spec[:, 161] = zeff
    spec[:, 162] = gam * f32(GC)
    spec[:, 164:325] = alphaS.reshape(ZMAX, NREF * NFREQ)
    return spec


def _build_program(s6, s8, a1, a2, debug=False):
    import concourse.bass as bass
    import concourse.bacc as bacc
    import concourse.mybir as mybir
    from concourse import tile

    f32 = mybir.dt.float32
    i32 = mybir.dt.int32
    u16 = mybir.dt.uint16
    Alu = mybir.AluOpType
    Act = mybir.ActivationFunctionType

    nc = bacc.Bacc(trn_type="TRN2", num_devices=M)
    r_in = nc.dram_tensor("r", [128, F], f32, kind="ExternalInput")
    o16_in = nc.dram_tensor("offs16", [128, F], u16, kind="ExternalInput")
    jtab_in = nc.dram_tensor("jtab", [NPAD_ALL, 2], f32, kind="ExternalInput")
    ctab_in = nc.dram_tensor("ctab", [128, AB * 4], f32, kind="ExternalInput")
    zoff_in = nc.dram_tensor("zoff", [128, AB], i32, kind="ExternalInput")
    spec_in = nc.dram_tensor("spec", [ZMAX, SREC], f32, kind="ExternalInput")
    e_out = nc.dram_tensor("eatom", [128, AB], f32, kind="ExternalOutput")
    if debug:
        cov_out = nc.dram_tensor("cov_out", [128, AB], f32, kind="ExternalOutput")
        A_out = nc.dram_tensor("A_out", [128, AB * 24], f32, kind="ExternalOutput")

    def view(t, off, dims):
        a = t[:]
        return bass.AP(
            tensor=a.tensor,
            offset=a.offset + off,
            ap=[list(a.ap[0])] + [list(d) for d in dims],
        )

    ln_den = float(np.log(0.5 * K4))
    isK6 = 1.0 / math.sqrt(K6)

    with tile.TileContext(nc) as tc:
        with (
            tc.tile_pool(name="io", bufs=1) as io,
            tc.tile_pool(name="dram", bufs=1, space="DRAM") as dpool,
        ):
            o16_t = io.tile([128, F], u16)
            offs_t = io.tile([128, F], i32)
            r_t = io.tile([128, F], f32)
            ctab_t = io.tile([128, AB * 4], f32)
            zoff_t = io.tile([128, AB], i32)
            A_sb = io.tile([128, AB * 24], f32)
            cov = io.tile([128, AB], f32)
            eat = io.tile([128, AB], f32)
            nc.sync.dma_start(out=o16_t[:], in_=o16_in[:])
            nc.sync.dma_start(out=r_t[:], in_=r_in[:])
            nc.sync.dma_start(out=ctab_t[:], in_=ctab_in[:])
            nc.sync.dma_start(out=zoff_t[:], in_=zoff_in[:])
            nc.vector.tensor_copy(out=offs_t[:], in_=o16_t[:])
            cKN = io.tile([128, 1], f32)
            cK5 = io.tile([128, 1], f32)
            cLD = io.tile([128, 1], f32)
            cGA = io.tile([128, 1], f32)
            nc.vector.memset(cKN[:], KN)
            nc.vector.memset(cK5[:], K5 * isK6)
            nc.vector.memset(cLD[:], ln_den)
            nc.vector.memset(cGA[:], GA)

            # ctab views [128, AB] (stride 4); bcast over 48 neighbors
            def cview(col, bcast=False):
                if bcast:
                    return view(ctab_t, col, [[4, AB], [0, N_NEIGH]])
                return view(ctab_t, col, [[4, AB]])

            # ---------------- Phase 1: coordination number ----------------
            with tc.tile_pool(name="s1", bufs=1) as s1:
                jrec = s1.tile([128, F * 2], f32)
                nc.gpsimd.indirect_dma_start(
                    out=jrec[:],
                    out_offset=None,
                    in_=jtab_in[:],
                    in_offset=bass.IndirectOffsetOnAxis(ap=offs_t[:], axis=0),
                )
                t1 = s1.tile([128, F], f32)
                t2 = s1.tile([128, F], f32)
                g3 = lambda t, off=0: view(t, off, [[48, AB], [1, N_NEIGH]])
                rcv = view(jrec, 0, [[96, AB], [2, N_NEIGH]])
                env = view(jrec, 1, [[96, AB], [2, N_NEIGH]])
                # rco = rcovK2_i + rcovK2_j ; m = r / rco
                nc.vector.tensor_tensor(out=g3(t1), in0=rcv, in1=cview(0, True), op=Alu.add)
                nc.vector.reciprocal(out=t1[:], in_=t1[:])
                nc.vector.tensor_tensor(out=t1[:], in0=t1[:], in1=r_t[:], op=Alu.mult)
                # E = erf(-KN*(m - 1))
                nc.scalar.activation(t1[:], t1[:], Act.Erf, bias=cKN[:], scale=-KN)
                # den' = 0.5*K4*exp(-((|en_i-en_j|+K5)^2)/K6)
                nc.vector.tensor_tensor(out=g3(t2), in0=cview(1, True), in1=env, op=Alu.subtract)
                nc.scalar.activation(t2[:], t2[:], Act.Abs)
                nc.scalar.activation(t2[:], t2[:], Act.Square, bias=cK5[:], scale=isK6)
                nc.scalar.activation(t2[:], t2[:], Act.Exp, bias=cLD[:], scale=-1.0)
                # tmp = (E + 1) * den' ; covcn = sum over 48 neighbors
                nc.vector.scalar_tensor_tensor(
                    out=t2[:], in0=t1[:], scalar=1.0, in1=t2[:],
                    op0=Alu.add, op1=Alu.mult,
                )
                nc.vector.tensor_reduce(
                    out=cov[:], in_=g3(t2), axis=mybir.AxisListType.X, op=Alu.add
                )

            # ---------------- Phase 2: gweights, zeta, A ----------------
            with tc.tile_pool(name="s2", bufs=1) as s2:
                prec = s2.tile([128, AB * SREC], f32)
                nc.gpsimd.indirect_dma_start(
                    out=prec[:],
                    out_offset=None,
                    in_=spec_in[:],
                    in_offset=bass.IndirectOffsetOnAxis(ap=zoff_t[:], axis=0),
                )
                g1 = s2.tile([128, AB * 49], f32)
                vg = view(g1, 0, [[49, AB], [7, NREF], [1, KCN]])
                nc.vector.tensor_tensor(
                    out=vg,
                    in0=view(cov, 0, [[1, AB], [0, NREF], [0, KCN]]),
                    in1=view(prec, 98, [[SREC, AB], [7, NREF], [1, KCN]]),
                    op=Alu.subtract,
                )
                nc.scalar.activation(g1[:], g1[:], Act.Square)
                nc.vector.tensor_tensor(
                    out=vg, in0=vg,
                    in1=view(prec, 49, [[SREC, AB], [7, NREF], [1, KCN]]),
                    op=Alu.mult,
                )
                nc.scalar.activation(g1[:], g1[:], Act.Exp)
                nc.vector.tensor_tensor(
                    out=vg, in0=vg,
                    in1=view(prec, 0, [[SREC, AB], [7, NREF], [1, KCN]]),
                    op=Alu.mult,
                )
                gw = s2.tile([128, AB * NREF], f32)
                vgw = view(gw, 0, [[NREF, AB], [1, NREF]])
                nc.vector.tensor_reduce(
                    out=vgw, in_=vg, axis=mybir.AxisListType.X, op=Alu.add
                )
                nrm = s2.tile([128, AB], f32)
                nc.vector.tensor_reduce(
                    out=nrm[:], in_=vgw, axis=mybir.AxisListType.X, op=Alu.add
                )
                mk = s2.tile([128, AB], f32)
                nc.vector.tensor_scalar(
                    out=mk[:], in0=nrm[:], scalar1=1e-8, scalar2=None, op0=Alu.is_gt
                )
                nc.vector.tensor_scalar_max(out=nrm[:], in0=nrm[:], scalar1=1e-8)
                nc.vector.reciprocal(out=nrm[:], in_=nrm[:])
                nc.vector.tensor_tensor(
                    out=vgw, in0=vgw, in1=view(nrm, 0, [[1, AB], [0, NREF]]),
                    op=Alu.mult,
                )
                # gfinal = (gwn - fixg)*mask + fixg   (mask is exactly 0/1)
                fixg_v = view(prec, 147, [[SREC, AB], [1, NREF]])
                nc.vector.tensor_tensor(out=vgw, in0=vgw, in1=fixg_v, op=Alu.subtract)
                nc.vector.tensor_tensor(
                    out=vgw, in0=vgw, in1=view(mk, 0, [[1, AB], [0, NREF]]),
                    op=Alu.mult,
                )
                nc.vector.tensor_tensor(out=vgw, in0=vgw, in1=fixg_v, op=Alu.add)
                # zeta
                qm = s2.tile([128, AB], f32)
                nc.vector.tensor_tensor(
                    out=qm[:], in0=view(prec, 161, [[SREC, AB]]),
                    in1=cview(3), op=Alu.add,
                )
                nc.vector.tensor_scalar_max(out=qm[:], in0=qm[:], scalar1=1e-8)
                nc.vector.reciprocal(out=qm[:], in_=qm[:])
                zt = s2.tile([128, AB * NREF], f32)
                vzt = view(zt, 0, [[NREF, AB], [1, NREF]])
                nc.vector.tensor_tensor(
                    out=vzt,
                    in0=view(prec, 161, [[SREC, AB], [0, NREF]]),
                    in1=view(prec, 154, [[SREC, AB], [1, NREF]]),
                    op=Alu.add,
                )
                nc.vector.tensor_tensor(
                    out=vzt, in0=vzt, in1=view(qm, 0, [[1, AB], [0, NREF]]),
                    op=Alu.mult,
                )
                nc.vector.tensor_scalar(
                    out=vzt, in0=vzt, scalar1=-1.0, scalar2=1.0,
                    op0=Alu.mult, op1=Alu.add,
                )
                nc.vector.tensor_tensor(
                    out=vzt, in0=vzt,
                    in1=view(prec, 162, [[SREC, AB], [0, NREF]]), op=Alu.mult,
                )
                nc.scalar.activation(zt[:], zt[:], Act.Exp)
                nc.scalar.activation(zt[:], zt[:], Act.Exp, bias=cGA[:], scale=-GA)
                nc.vector.tensor_tensor(out=vzt, in0=vzt, in1=vgw, op=Alu.mult)
                # A[x] = sum_a zeta[a] * alphaS[a, x]
                pa = s2.tile([128, AB * NREF * NFREQ], f32)
                vpa = view(pa, 0, [[161, AB], [7, NFREQ], [1, NREF]])
                nc.vector.tensor_tensor(
                    out=vpa,
                    in0=view(prec, 164, [[SREC, AB], [1, NFREQ], [NFREQ, NREF]]),
                    in1=view(zt, 0, [[NREF, AB], [0, NFREQ], [1, NREF]]),
                    op=Alu.mult,
                )
                nc.vector.tensor_reduce(
                    out=view(A_sb, 0, [[24, AB], [1, NFREQ]]),
                    in_=vpa, axis=mybir.AxisListType.X, op=Alu.add,
                )
                nc.vector.tensor_copy(
                    view(A_sb, 23, [[24, AB]]), cview(2)
                )

            # ---------------- AllGather A ----------------
            Aloc = dpool.tile([NPAD, 24], f32)
            Afull = dpool.tile([NPAD_ALL, 24], f32)
            nc.sync.dma_start(
                out=Aloc[:].rearrange("(p a) x -> p a x", p=128),
                in_=view(A_sb, 0, [[24, AB], [1, 24]]),
            )
            nc.gpsimd.collective_compute(
                "AllGather",
                mybir.AluOpType.bypass,
                replica_groups=[list(range(M))],
                ins=[Aloc[:].opt()],
                outs=[Afull[:].opt()],
            )

            # ---------------- Phase 3: pair energies ----------------
            sqrt3 = math.sqrt(3.0)
            with tc.tile_pool(name="s3", bufs=2) as s3:
                for k in range(NCHUNK):
                    o = k * FC
                    jr = s3.tile([128, FC * 24], f32, tag="jr")
                    nc.gpsimd.indirect_dma_start(
                        out=jr[:],
                        out_offset=None,
                        in_=Afull[:],
                        in_offset=bass.IndirectOffsetOnAxis(
                            ap=offs_t[:, o:o + FC], axis=0
                        ),
                    )
                    pr = s3.tile([128, FC * NFREQ], f32, tag="pr")
                    vpr = view(pr, 0, [[1104, CH], [NFREQ, N_NEIGH], [1, NFREQ]])
                    nc.vector.tensor_tensor(
                        out=vpr,
                        in0=view(jr, 0, [[1152, CH], [24, N_NEIGH], [1, NFREQ]]),
                        in1=view(A_sb, 24 * CH * k, [[24, CH], [0, N_NEIGH], [1, NFREQ]]),
                        op=Alu.mult,
                    )
                    c6 = s3.tile([128, FC], f32, tag="c6")
                    v3 = lambda t: view(t, 0, [[48, CH], [1, N_NEIGH]])
                    nc.vector.tensor_reduce(
                        out=v3(c6), in_=vpr, axis=mybir.AxisListType.X, op=Alu.add
                    )
                    rch = view(r_t, o, [[48, CH], [1, N_NEIGH]])
                    q2 = s3.tile([128, FC], f32, tag="q2")
                    q4 = s3.tile([128, FC], f32, tag="q4")
                    q6 = s3.tile([128, FC], f32, tag="q6")
                    q8 = s3.tile([128, FC], f32, tag="q8")
                    rr = s3.tile([128, FC], f32, tag="rr")
                    o2 = s3.tile([128, FC], f32, tag="o2")
                    nc.scalar.activation(v3(q2), rch, Act.Square)
                    nc.scalar.activation(q4[:], q2[:], Act.Square)
                    nc.vector.tensor_tensor(
                        out=q6[:], in0=q2[:], in1=q4[:], op=Alu.mult
                    )
                    nc.scalar.activation(q8[:], q4[:], Act.Square)
                    # r4r2 = (s_j*sqrt3)*s_i ; r0 = a1*r4r2 + a2
                    nc.vector.scalar_tensor_tensor(
                        out=v3(rr),
                        in0=view(jr, 23, [[1152, CH], [24, N_NEIGH]]),
                        scalar=sqrt3,
                        in1=view(ctab_t, 4 * CH * k + 2, [[4, CH], [0, N_NEIGH]]),
                        op0=Alu.mult, op1=Alu.mult,
                    )
                    nc.vector.tensor_scalar(
                        out=o2[:], in0=rr[:], scalar1=float(a1), scalar2=float(a2),
                        op0=Alu.mult, op1=Alu.add,
                    )
                    nc.scalar.activation(o2[:], o2[:], Act.Square)
                    o4 = q4  # reuse
                    nc.scalar.activation(o4[:], o2[:], Act.Square)
                    nc.vector.tensor_tensor(
                        out=o2[:], in0=o2[:], in1=o4[:], op=Alu.mult
                    )  # r0^6
                    nc.vector.tensor_tensor(
                        out=q6[:], in0=q6[:], in1=o2[:], op=Alu.add
                    )
                    nc.scalar.activation(o4[:], o4[:], Act.Square)  # r0^8
                    nc.vector.tensor_tensor(
                        out=q8[:], in0=q8[:], in1=o4[:], op=Alu.add
                    )
                    nc.vector.reciprocal_approx_fast(out=q6[:], in_=q6[:])
                    nc.vector.reciprocal_approx_fast(out=q8[:], in_=q8[:])
                    nc.scalar.activation(rr[:], rr[:], Act.Square)  # r4r2^2
                    nc.vector.tensor_tensor(
                        out=rr[:], in0=rr[:], in1=q8[:], op=Alu.mult
                    )
                    nc.vector.scalar_tensor_tensor(
                        out=rr[:], in0=rr[:], scalar=float(s8 / s6), in1=q6[:],
                        op0=Alu.mult, op1=Alu.add,
                    )
                    nc.vector.tensor_tensor(
                        out=rr[:], in0=rr[:], in1=c6[:], op=Alu.mult
                    )
                    nc.vector.tensor_reduce(
                        out=eat[:, k * CH:(k + 1) * CH], in_=v3(rr),
                        axis=mybir.AxisListType.X, op=Alu.add,
                    )
            if debug:
                nc.sync.dma_start(out=cov_out[:], in_=cov[:])
                nc.sync.dma_start(out=A_out[:], in_=A_sb[:])
            nc.vector.tensor_scalar_mul(
                out=eat[:], in0=eat[:], scalar1=float(-C2EV * s6)
            )
            nc.sync.dma_start(out=e_out[:], in_=eat[:])
    nc.compile()
    return nc


def _host_fallback(inp):
    """Pure-numpy reference path (used only if idx_i lacks block structure)."""
    f32 = np.float32
    from numpy import exp, abs as nabs

    def erf_np(x):
        try:
            from scipy.special import erf
            return erf(x).astype(f32)
        except Exception:
            import math as m
            return np.vectorize(m.erf, otypes=[f32])(x)

    Z = np.asarray(inp["Z"]).astype(np.int64)
    idx_i = np.asarray(inp["idx_i"]).astype(np.int64)
    idx_j = np.asarray(inp["idx_j"]).astype(np.int64)
    r = np.asarray(inp["r_ij"], f32) * f32(C2BOHR)
    qa = np.asarray(inp["qa"], f32)
    n = qa.shape[0]
    scaleq = _sp(inp["scaleq_raw"])
    s6 = _sp(inp["s6_raw"]); s8 = _sp(inp["s8_raw"])
    a1 = _sp(inp["a1_raw"]); a2 = _sp(inp["a2_raw"])
    spec = _host_tables(inp)
    alphaS = spec[:, 164:325].reshape(ZMAX, NREF, NFREQ)
    rcov = np.asarray(inp["rcov"], f32); en = np.asarray(inp["en"], f32)
    Zi = Z[idx_i]; Zj = Z[idx_j]
    rco = f32(K2) * (rcov[Zi] + rcov[Zj])
    den = f32(K4) * exp(-((nabs(en[Zi] - en[Zj]) + f32(K5)) ** 2) / f32(K6))
    tmp = den * f32(0.5) * (1.0 + erf_np(-KN * (r - rco) / rco))
    covcn = np.zeros(n, f32)
    np.add.at(covcn, idx_i, tmp)
    nm = np.asarray(inp["ncount_mask"], f32)
    nw = np.asarray(inp["ncount_weight"], f32)
    cn = np.asarray(inp["cn"], f32)
    gw = np.sum(nm[Z] * exp(-WF * nw[Z] * (covcn[:, None, None] - cn[Z]) ** 2), -1)
    nrm = gw.sum(-1, keepdims=True)
    gw = np.where(nrm > 1e-8, gw / np.where(nrm > 1e-8, nrm, 1), np.asarray(inp["fixgweights"], f32)[Z])
    zeff = np.asarray(inp["zeff"], f32); gam = np.asarray(inp["gam"], f32)
    iz = zeff[Z][:, None]
    qref = iz + np.asarray(inp["refq"], f32)[Z] * scaleq
    qmod = iz + qa[:, None]
    qmod_ = np.where(qmod > 1e-8, qmod, 1.0)
    zeta = np.where(qmod > 1e-8,
                    exp(GA * (1.0 - exp(gam[Z][:, None] * GC * (1.0 - qref / qmod_)))),
                    f32(math.exp(GA))) * gw
    A = np.einsum("na,nax->nx", zeta.astype(f32), alphaS[Z]).astype(f32)
    c6 = np.einsum("px,px->p", A[idx_i], A[idx_j]).astype(f32)
    sq = np.asarray(inp["sqrt_r4r2"], f32)
    r4r2 = f32(math.sqrt(3.0)) * sq[Zi] * sq[Zj]
    r0 = a1 * r4r2 + a2
    oor6 = 1.0 / (r ** 6 + r0 ** 6)
    oor8 = 1.0 / (r ** 8 + r0 ** 8)
    ed = -c6 * (s6 * oor6 + s8 * r4r2 ** 2 * oor8) * f32(C2EV)
    eatom = np.zeros(n, f32)
    np.add.at(eatom, idx_i, ed.astype(f32))
    z = np.zeros(n, f32)
    return eatom.astype(f32), z, z


def kernel(**inputs):
    global LAST_RESULTS
    f32 = np.float32
    inp = {k: np.asarray(v) for k, v in inputs.items()}
    idx_i = inp["idx_i"].astype(np.int64)
    if not np.array_equal(idx_i, np.repeat(np.arange(N_ATOMS, dtype=np.int64), N_NEIGH)):
        return _host_fallback(inp)

    from concourse import bass_utils

    Z = inp["Z"].astype(np.int64)
    idx_j = inp["idx_j"].astype(np.int64)
    r = (inp["r_ij"].astype(f32) * f32(C2BOHR)).reshape(N_ATOMS, N_NEIGH)
    qa = inp["qa"].astype(f32)
    s6 = float(_sp(inp["s6_raw"])); s8 = float(_sp(inp["s8_raw"]))
    a1 = float(_sp(inp["a1_raw"])); a2 = float(_sp(inp["a2_raw"]))

    spec = _host_tables(inp)
    rcovK2 = (f32(K2) * inp["rcov"].astype(f32))[Z]      # [N] per-atom
    en_a = inp["en"].astype(f32)[Z]
    r4_a = inp["sqrt_r4r2"].astype(f32)[Z]

    gidx = ((idx_j // NLOC) * NPAD + (idx_j % NLOC)).astype(np.uint16)
    gidx = gidx.reshape(N_ATOMS, N_NEIGH)

    # replicated per-atom j-side table in padded-global layout
    jtab = np.ones((NPAD_ALL, 2), f32)
    for c in range(M):
        sl = slice(c * NLOC, (c + 1) * NLOC)
        jtab[c * NPAD:c * NPAD + NLOC, 0] = rcovK2[sl]
        jtab[c * NPAD:c * NPAD + NLOC, 1] = en_a[sl]

    key = (s6, s8, a1, a2)
    import os
    dbg = bool(int(os.environ.get("KERNEL_DEBUG", "0")))
    key = key + (dbg,)
    if key not in _CACHE:
        _CACHE.clear()
        _CACHE[key] = _build_program(s6, s8, a1, a2, debug=dbg)
    nc = _CACHE[key]

    in_maps = []
    for c in range(M):
        sl = slice(c * NLOC, (c + 1) * NLOC)
        rp = np.full((NPAD, N_NEIGH), 3.0, f32)
        rp[:NLOC] = r[sl]
        op = np.zeros((NPAD, N_NEIGH), np.uint16)
        op[:NLOC] = gidx[sl]
        ct = np.zeros((NPAD, 4), f32)
        ct[:NLOC, 0] = rcovK2[sl]; ct[NLOC:, 0] = 1.0
        ct[:NLOC, 1] = en_a[sl]; ct[NLOC:, 1] = 1.0
        ct[:NLOC, 2] = r4_a[sl]; ct[NLOC:, 2] = 1.0
        ct[:NLOC, 3] = qa[sl]
        zo = np.zeros(NPAD, np.int32)
        zo[:NLOC] = Z[sl]
        in_maps.append({
            "r": rp.reshape(128, F),
            "offs16": op.reshape(128, F),
            "ctab": ct.reshape(128, AB * 4),
            "zoff": zo.reshape(128, AB),
            "jtab": jtab,
            "spec": spec,
        })

    import time as _time
    _t0 = _time.time()
    res = bass_utils.run_bass_kernel_spmd(nc, in_maps, core_ids=list(range(M)))
    global LAST_RUN_S
    LAST_RUN_S = _time.time() - _t0
    LAST_RESULTS = res
    eatom = np.zeros(N_ATOMS, f32)
    for c in range(M):
        e = np.asarray(res.results[c]["eatom"], f32).reshape(NPAD)
        eatom[c * NLOC:(c + 1) * NLOC] = e[:NLOC]
    z = np.zeros(N_ATOMS, f32)
    return eatom, z, z
